# revision 21
# baseline (speedup 1.0000x reference)
"""SPRING subsequence-DTW (32-tap kernel over a 2^22 stream) on 8 trn2 cores.

Strategy: the length-n stream is cut into 1024 segments of 4096 columns, each
with a 64-column left halo (max optimal-path span over all eps-candidates is 61
columns, so a 64-column halo reproduces the full DP exactly on owned columns).
Each core gets 128 segments, one per SBUF partition.

The whole row recurrence D[i,t] = min(D[i,t-1], D[i-1,t], D[i-1,t-1]) + d[i,t]
runs as ONE custom DVE instruction per row (SPRING_ROW_FULL_ANT, registered
below) at 1 cycle/element -- 3x the stock tensor_tensor(min) +
tensor_tensor_scan pair.  It computes, per element,
    diag[t] = up[t-1]                  (one-element delayed tap of Src0 via a
                                        CURR_ALU_OUT lane capture; seeded from
                                        the s1 scalar at the first element)
    c[t]  = min(up[t], diag[t])
    P[t]  = P[t-1] + d[t]              (ADD scan, init 0)
    A[t]  = c[t] - P[t] + d[t]         (= c[t] - P[t-1])
    M[t]  = min(M[t-1], A[t])          (MIN scan, init s0 = D[lo-1] carry)
    out[t] = P[t] + M[t]               (= D[i, t])
which is the closed form of the (min,+) row scan; the two one-stage feedback
scans have no pipeline bubble.  (x - k_i)^2 is produced by the Scalar engine
in parallel.  D rows ping-pong between two SBUF buffers whose column 0 holds
an INF sentinel, so every chunk's carries are plain [P,1] views: s0 =
out_buf[:, lo:lo+1], s1 = in_buf[:, lo:lo+1].  The kernel returns the last DP
row; the tiny finale (top-30 endpoint selection, start-column backtrack,
interval painting) runs on host.
"""

import numpy as np

N = 4194304
KERNEL_LEN = 32
EPS = 0.5
MAX_PATH = 30
NCORES = 8
P = 128
SEG = 4096
HALO = 64
LH = SEG + HALO  # 4160
# Left-pad sentinel: (PAD_X - k)^2 >= ~5.8 >> EPS kills paths into the pad
# while keeping the closed-form cumsum P small (fp32 cancellation stays tiny).
PAD_X = 3.0
INF = 3.0e38

_CACHE: dict = {}

OP_NAME = "SPRING_ROW_FULL_ANT"


def _register_spring_op():
    """Register the fused SPRING row-step custom DVE op (see module docstring).

    Built from the documented Spec DSL with two hand patches the DSL cannot
    express: (a) the nested-scan check in Scan.__post_init__ is bypassed (the
    scheduler places the two one-stage feedback recurrences on separate
    stages); (b) the diagonal is a one-element-delayed tap of Src0 -- a
    CURR_ALU_OUT lane capture at stage 0, read by the MIN at stage 1, seeded
    from the s1 scalar by the seed uOp.  Both patches are HW-verified.
    """
    import concourse.dve_ops as dve_ops
    from concourse.dve_spec import Spec, Src0, Src1, C0, AluOp, Bin, Scan, lower
    from concourse.dve_uop import DveOpSpec, AluInp, DelayInp, InpSel, ENABLE

    if OP_NAME in dve_ops._SUB_OPCODE_FOR_NAME:
        return next(o for o in dve_ops.OPS if o.name == OP_NAME)

    def mk_scan(op, expr, init=None):
        s = object.__new__(Scan)
        for k, v in dict(op=op, expr=expr, init=init, _subdim_step=None).items():
            object.__setattr__(s, k, v)
        return s

    bypass = Bin(AluOp.BYPASS, Src0, Src0)  # becomes the delayed tap
    c = Bin(AluOp.MIN, Src0, bypass)
    Pn = mk_scan(AluOp.ADD, Src1)
    A = Bin(AluOp.ADD, Bin(AluOp.SUBTRACT, c, Pn), Src1)
    M = mk_scan(AluOp.MIN, A, C0)
    body = Bin(AluOp.ADD, Pn, M)

    def reference(in0, in1, s0, s1, imm2):
        u = in0.astype(np.float32)
        d = in1.astype(np.float32)
        st = np.broadcast_to(np.asarray(s0, np.float32), u.shape[:-1]).copy()
        diag = np.broadcast_to(np.asarray(s1, np.float32), u.shape[:-1]).copy()
        out = np.empty_like(u)
        for t in range(u.shape[-1]):
            cc = np.minimum(u[..., t], diag)
            st = (np.minimum(cc, st) + d[..., t]).astype(np.float32)
            out[..., t] = st
            diag = u[..., t]
        return out

    spec = Spec(body=body, reference=reference)
    uops = lower(spec, ver="v3")
    seed, steady = uops
    # Seed: route CONST_1 (the s1 scalar) into input slot 5 -- the 5th enabled
    # slot, i.e. delay lane 4 -- and have stage 0 emit it, so the first steady
    # element's delayed tap reads the diagonal carry.
    seed.inp[5] = InpSel.CONST_1
    seed.inp_enable[5] = 1
    lane4 = AluInp(int(AluInp.PREV_DELAY_0) + 4)
    seed.datapath_config[0].alu_src0 = lane4
    seed.datapath_config[0].alu_src1 = lane4
    # Steady: lane 4 captures stage 0's own previous-element output (the
    # one-element delay); the MIN at stage 1 reads it as the diagonal.
    steady.datapath_config[0].delay[4] = DelayInp.CURR_ALU_OUT
    steady.datapath_config[0].delay_enable[4] = ENABLE
    steady.datapath_config[1].alu_src1 = lane4

    class _HandDveOp:
        name = OP_NAME
        subdim = False
        perf_en: dict = {}

        def __init__(self):
            self.spec = spec

        def compile(self, ver):
            from concourse.dve_ops import get_dve_sub_opcode

            return DveOpSpec(
                name=OP_NAME,
                opcode=get_dve_sub_opcode(OP_NAME),
                uops=uops,
                rd1_en=True,
            )

    op = _HandDveOp()
    row = max(dve_ops._SUB_OPCODE_FOR_NAME.values()) + 1
    assert row < 0x20
    dve_ops.OPS.append(op)
    dve_ops._SUB_OPCODE_FOR_NAME[OP_NAME] = row
    dve_ops.CUSTOM_DVE_SPECS[OP_NAME] = spec
    return op


OP1_NAME = "SPRING_ROW1_FUSED_ANT"


def _register_spring_row1_op():
    """Row-1 variant that also squares the raw x stream into row 0's D:
        up[t] = (x[t] + kneg0)^2   (kneg0 via s0/C0; = D[0, t])
        c[t]  = min(up[t], up[t-1])  (delayed tap, seeded from s1/C1)
        then the same closed-form (min,+) scan; M-init comes from imm2/C2
        (row 1 always runs full-width, so the init is the INF constant).
    Uses all 8 ALU stages.  Removes the row-0 Square pass from the critical
    path: row 1 consumes x directly, D[0] is never materialized.
    """
    import concourse.dve_ops as dve_ops
    from concourse.dve_spec import Spec, Src0, Src1, C0, C2, AluOp, Bin, Scan, lower
    from concourse.dve_uop import DveOpSpec, AluInp, DelayInp, InpSel, ENABLE

    if OP1_NAME in dve_ops._SUB_OPCODE_FOR_NAME:
        return next(o for o in dve_ops.OPS if o.name == OP1_NAME)

    def mk_scan(op, expr, init=None):
        s = object.__new__(Scan)
        for k, v in dict(op=op, expr=expr, init=init, _subdim_step=None).items():
            object.__setattr__(s, k, v)
        return s

    t = Bin(AluOp.ADD, Src0, C0)
    up = Bin(AluOp.MULTIPLY, t, t)
    c = Bin(AluOp.MIN, up, up)      # second operand becomes the delayed tap
    Pn = mk_scan(AluOp.ADD, Src1)
    A = Bin(AluOp.ADD, Bin(AluOp.SUBTRACT, c, Pn), Src1)
    M = mk_scan(AluOp.MIN, A, C2)
    body = Bin(AluOp.ADD, Pn, M)

    def reference(in0, in1, s0, s1, imm2):
        u = (in0.astype(np.float32) + np.asarray(s0, np.float32)[..., None]) ** 2
        d = in1.astype(np.float32)
        st = np.full(u.shape[:-1], np.float32(imm2), np.float32)
        diag = np.broadcast_to(np.asarray(s1, np.float32), u.shape[:-1]).copy()
        out = np.empty_like(u)
        for k in range(u.shape[-1]):
            cc = np.minimum(u[..., k], diag)
            st = (np.minimum(cc, st) + d[..., k]).astype(np.float32)
            out[..., k] = st
            diag = u[..., k]
        return out

    spec = Spec(body=body, reference=reference)
    uops = lower(spec, ver="v3")
    seed, steady = uops
    # Enabled input slots are 1,2,3,4,5 -> lanes 0..4; lane 5 is free.
    lane5 = AluInp(int(AluInp.PREV_DELAY_0) + 5)
    # Seed: route CONST_1 (s1, the diag seed) into slot 6 -> lane 5, and have
    # the `up` stage (st1) emit it so element 0's tap reads it.
    seed.inp[6] = InpSel.CONST_1
    seed.inp_enable[6] = 1
    seed.datapath_config[0].pass_through_delay(5)
    seed.datapath_config[1].alu_src0 = lane5
    seed.datapath_config[1].alu_src1 = lane5
    seed.datapath_config[1].op = AluOp.BYPASS
    # Steady: lane 5 captures the `up` stage's previous-element output; the
    # MIN at st2 reads it as the diagonal.
    steady.datapath_config[1].delay[5] = DelayInp.CURR_ALU_OUT
    steady.datapath_config[1].delay_enable[5] = ENABLE
    steady.datapath_config[2].alu_src1 = lane5

    class _HandDveOp:
        name = OP1_NAME
        subdim = False
        perf_en: dict = {}

        def __init__(self):
            self.spec = spec

        def compile(self, ver):
            from concourse.dve_ops import get_dve_sub_opcode

            return DveOpSpec(
                name=OP1_NAME,
                opcode=get_dve_sub_opcode(OP1_NAME),
                uops=uops,
                rd1_en=True,
            )

    op = _HandDveOp()
    row = max(dve_ops._SUB_OPCODE_FOR_NAME.values()) + 1
    assert row < 0x20
    dve_ops.OPS.append(op)
    dve_ops._SUB_OPCODE_FOR_NAME[OP1_NAME] = row
    dve_ops.CUSTOM_DVE_SPECS[OP1_NAME] = spec
    return op


def _build():
    import concourse.bacc as bacc
    import concourse.mybir as mybir
    from concourse.tile import TileContext

    spring_op = _register_spring_op()
    spring1_op = _register_spring_row1_op()

    nc = bacc.Bacc("TRN2", debug=False, num_devices=NCORES)
    x_d = nc.dram_tensor("x_seg", [P, LH], mybir.dt.float32, kind="ExternalInput")
    kb_d = nc.dram_tensor("kneg", [P, KERNEL_LEN], mybir.dt.float32, kind="ExternalInput")
    out_d = nc.dram_tensor("d_last", [P, SEG], mybir.dt.float32, kind="ExternalOutput")

    FT = mybir.ActivationFunctionType

    with TileContext(nc) as tc:
        with tc.tile_pool(name="main", bufs=1) as pool, tc.tile_pool(name="dbuf", bufs=3) as dpool:
            x_t = pool.tile([P, LH], mybir.dt.float32)
            kb_t = pool.tile([P, KERNEL_LEN], mybir.dt.float32)
            # Ping-pong D row buffers with a leading INF sentinel column:
            # index 1+j holds D[row, j]; index 0 stays INF so every chunk's
            # carries are the uniform [P,1] views s0/s1 = buf[:, lo:lo+1].
            DpA = pool.tile([P, 1 + LH], mybir.dt.float32)
            DpB = pool.tile([P, 1 + LH], mybir.dt.float32)

            # kb first (tiny, gates the first Act square), then the x chunks.
            # The sentinel column buf[:, 0] is never read (chunks starting at
            # lo=0 pass immediate INF carries), so no memset is needed.
            nc.sync.dma_start(kb_t[:, :], kb_d.ap())
            hb = [(0, 1152), (1152, 2304), (2304, 3456), (3456, 3968),
                  (3968, LH)]
            # Spread the x chunks across four DGE queues (queue = issuing
            # engine) so the transfers run on disjoint DMA engine sets.
            dma_engines = [nc.gpsimd, nc.scalar, nc.sync]
            for j, (lo, hi) in enumerate(hb):
                dma_engines[j % len(dma_engines)].dma_start(
                    x_t[:, lo:hi], x_d.ap()[:, lo:hi])

            def fused_row(Po, Pi, d_t, lo, hi):
                # Chunks starting at 0 have both carries equal to the INF
                # sentinel -- pass immediates to skip two AP reads.
                nc.vector._custom_dve(
                    spring_op,
                    out=Po[:, 1 + lo:1 + hi],
                    in0=Pi[:, 1 + lo:1 + hi],
                    in1=d_t[:, lo:hi],
                    s0=INF if lo == 0 else Po[:, lo:lo + 1],
                    s1=INF if lo == 0 else Pi[:, lo:lo + 1],
                )

            half = [(0, LH // 2), (LH // 2, LH)]
            tail = [(0, 1040), (1040, 2080), (2080, 3120), (3120, 4032),
                    (4032, LH)]
            for i in range(1, KERNEL_LEN):
                Pi = DpA if i % 2 == 1 else DpB   # holds row i-1
                Po = DpB if i % 2 == 1 else DpA   # receives row i
                d_t = dpool.tile([P, LH], mybir.dt.float32, tag="d")
                if i == 1:
                    # Head: Act computes d_1 per DMA chunk; one full-width
                    # row-1 fused op consumes raw x (squaring row 0 inside),
                    # so no row-0 square pass exists at all.
                    for lo, hi in hb:
                        nc.scalar.activation(d_t[:, lo:hi], x_t[:, lo:hi], FT.Square,
                                             bias=kb_t[:, i:i + 1], scale=1.0)
                    nc.vector._custom_dve(
                        spring1_op,
                        out=Po[:, 1:1 + LH],
                        in0=x_t[:, :],
                        in1=d_t[:, :],
                        s0=kb_t[:, 0:1],
                        s1=INF,
                        imm2=INF,
                    )
                    continue
                if i in (2, 3):
                    # Smooth the head->steady transition: halve rows 2-3 so
                    # the scan starts as soon as half of d is ready.
                    for lo, hi in half:
                        nc.scalar.activation(d_t[:, lo:hi], x_t[:, lo:hi], FT.Square,
                                             bias=kb_t[:, i:i + 1], scale=1.0)
                        fused_row(Po, Pi, d_t, lo, hi)
                    continue
                nc.scalar.activation(d_t[:, :], x_t[:, :], FT.Square,
                                     bias=kb_t[:, i:i + 1], scale=1.0)
                if i == KERNEL_LEN - 1:
                    # Tail: chunk the last row scan and DMA each output chunk
                    # as soon as it is written.  Alternate DGE queues -- on a
                    # single queue the ~2MB of output drains slower than the
                    # chunks are produced and the NEFF end waits on the
                    # backlog.
                    for j, (lo, hi) in enumerate(tail):
                        fused_row(Po, Pi, d_t, lo, hi)
                        olo, ohi = max(lo - HALO, 0), hi - HALO
                        eng = [nc.sync, nc.gpsimd, nc.scalar][j % 3]
                        eng.dma_start(out_d.ap()[:, olo:ohi],
                                      Po[:, 1 + max(lo, HALO):1 + hi])
                else:
                    fused_row(Po, Pi, d_t, 0, LH)
    nc.compile()
    return nc


def _get_nc():
    if "nc" not in _CACHE:
        _CACHE["nc"] = _build()
    return _CACHE["nc"]


def _run_device(x, k, trace=False):
    from concourse.bass_utils import run_bass_kernel_spmd

    nc = _get_nc()
    xp = np.concatenate([np.full(HALO, PAD_X, np.float32), x.astype(np.float32)])
    segs = np.lib.stride_tricks.sliding_window_view(xp, LH)[::SEG]
    segs = segs.reshape(NCORES, P, LH)
    kneg = np.ascontiguousarray(np.broadcast_to(-k.astype(np.float32), (P, KERNEL_LEN)))
    in_maps = [{"x_seg": np.ascontiguousarray(segs[c]), "kneg": kneg}
               for c in range(NCORES)]
    res = run_bass_kernel_spmd(nc, in_maps, core_ids=list(range(NCORES)), trace=trace)
    D = np.concatenate([res.results[c]["d_last"].reshape(-1) for c in range(NCORES)])
    return D, res


def _backtrack_start(x64, k64, e, W=256):
    """Start column of the optimal path ending at e (f64 windowed DP)."""
    w0 = max(0, e - W)
    xx = x64[w0:e + 1]
    m = xx.shape[0]
    D = (k64[0] - xx) ** 2
    S = np.arange(w0, e + 1)
    idx = np.arange(m)
    for i in range(1, KERNEL_LEN):
        d = (k64[i] - xx) ** 2
        D_sh = np.empty_like(D); D_sh[0] = 1e300; D_sh[1:] = D[:-1]
        S_sh = np.empty_like(S); S_sh[0] = S[0]; S_sh[1:] = S[:-1]
        td = D_sh < D
        c = np.where(td, D_sh, D)
        cs = np.where(td, S_sh, S)
        Pc = np.cumsum(d)
        a = c - (Pc - d)
        mv = np.minimum.accumulate(a)
        upd = np.empty(m, dtype=bool); upd[0] = True
        upd[1:] = a[1:] < mv[:-1]
        pos = np.maximum.accumulate(np.where(upd, idx, 0))
        D = Pc + mv
        S = cs[pos]
    return int(S[-1])


def _finalize(D, x, k):
    part = np.argpartition(D, MAX_PATH)[:MAX_PATH + 1]
    order = part[np.argsort(D[part], kind="stable")][:MAX_PATH]
    # argpartition ties at the boundary: fall back to exact stable order among
    # the partitioned candidates extended by any equal-valued columns
    thr = D[order[-1]]
    if (D <= thr).sum() > MAX_PATH:
        cand = np.where(D <= thr)[0]
        order = cand[np.argsort(D[cand], kind="stable")][:MAX_PATH]
    sel = order[D[order] <= EPS]
    out = np.zeros(N, dtype=np.float32)
    if sel.size == 0:
        return out
    x64 = x.astype(np.float64)
    k64 = k.astype(np.float64)
    # paint from worst to best so the smallest cost wins overlaps
    sel = sel[np.argsort(D[sel], kind="stable")]
    for e in sel[::-1]:
        s = _backtrack_start(x64, k64, int(e))
        out[s:e] = D[e]
    return out


def kernel(x, kernel):
    x = np.asarray(x, dtype=np.float32)
    k = np.asarray(kernel, dtype=np.float32)
    assert x.shape == (N,) and k.shape == (KERNEL_LEN,)
    D, _ = _run_device(x, k)
    return _finalize(D, x, k)
